# revision 9
# baseline (speedup 1.0000x reference)
"""Trainium2 Bass kernel for ChunkedGeoSparseLinear (gather-mode sparse linear).

out[n, o] = sum_k x[n, idx[o, k]] * w[o, k] + b[o]
  x: (4096, 4096) f32, idx: (4096, 16) i64, w: (4096, 16) f32, b: (4096,) f32

Strategy (8 cores = 4 row-slabs x 2 output-halves):
  - Host: transpose x -> xT, cast to bf16. Core d = (slab s=d%4, half h=d//4)
    gets xT[:, 1024s:1024s+1024] (2 KiB rows -> bigger DMA descriptors) and
    the idx/weight/bias tables for outputs [2048h, 2048h+2048).
  - Device: dma_gather (SWDGE, 4 queues) pulls the 32768 tap rows
    xT[idx[o,k], :] from HBM into SBUF tiles of 128 taps x 1024 cols. Taps are
    pre-ordered so tile (G, m) holds tap m of group G's 128 outputs.
  - PE: per output group G, 16 taps x 2 column-halves of matmuls with
    *diagonal* lhsT (diag of w[:, m]) accumulate into one PSUM tile
    [128 outputs, 1024 rows] (2 banks, one per 512-wide matmul).
  - ScalarE drains PSUM with the bias add; DMA writes the outT slab; host
    transposes and stitches the 4x2 grid back together.
"""

import numpy as np
import ml_dtypes

N = 4096
IN_F = 4096
OUT_F = 4096
K = 16
NCORES = 8
NSPLIT = 4                    # row-slab splits
OSPLIT = 2                    # output-half splits
NSLAB = N // NSPLIT           # 1024 rows per core
OSLAB = OUT_F // OSPLIT       # 2048 outputs per core
NGRP = OSLAB // 128           # 16 psum groups of 128 outputs
TAPS = OSLAB * K              # 32768 taps per core
TPC = 1024                    # taps per dma_gather call (SWDGE ring holds 1024
                              # descriptors; >1024-idx gathers crash the device)
CPG = (128 * K) // TPC        # gather calls per psum group (2)

_CACHE = {}


def _build(reps: int = 1):
    """Build + compile the per-core Bass program (SPMD: same program, 8 cores)."""
    import concourse.bacc as bacc
    import concourse.mybir as mybir
    import concourse.tile as tile

    dt = mybir.dt
    nc = bacc.Bacc("TRN2", debug=False, num_devices=NCORES,
                   enable_partition_id=False, num_swdge_queues=4)

    xt = nc.dram_tensor("xt", [IN_F, NSLAB], dt.bfloat16, kind="ExternalInput")
    idxs = nc.dram_tensor("idxs", [128, TAPS // 16], dt.int16, kind="ExternalInput")
    wcol = nc.dram_tensor("wcol", [128, TAPS // 128], dt.bfloat16, kind="ExternalInput")
    bias = nc.dram_tensor("bias", [128, NGRP], dt.float32, kind="ExternalInput")
    ident_d = nc.dram_tensor("ident", [128, 128], dt.bfloat16, kind="ExternalInput")
    outT = nc.dram_tensor("outT", [OSLAB, NSLAB], dt.float32, kind="ExternalOutput")
    # reps-dependent output shape keeps timing variants from aliasing in the
    # executable cache (the cache key ignores the embedded BIR)
    nc.dram_tensor("repstag", [1, reps], dt.float32, kind="ExternalOutput")

    with tile.TileContext(nc) as tc:
        with (
            tc.tile_pool(name="singles", bufs=1) as singles,
            tc.tile_pool(name="gpool", bufs=10) as gpool,
            tc.tile_pool(name="dpool", bufs=3) as dpool,
            tc.tile_pool(name="ppool", bufs=4, space="PSUM") as ppool,
            tc.tile_pool(name="opool", bufs=4) as opool,
        ):
            idxs_sb = singles.tile([128, TAPS // 16], dt.int16)
            nc.sync.dma_start(idxs_sb[:], idxs[:])
            w_sb = singles.tile([128, TAPS // 128], dt.bfloat16)
            nc.sync.dma_start(w_sb[:], wcol[:])
            bias_sb = singles.tile([128, NGRP], dt.float32)
            nc.sync.dma_start(bias_sb[:], bias[:])
            ident = singles.tile([128, 128], dt.bfloat16)
            nc.sync.dma_start(ident[:], ident_d[:])

            def body(_i=None):
                ident_b = ident[:].unsqueeze(1).broadcast_to([128, K, 128])
                tiles_per_call = TPC // 128  # 8
                for G in range(NGRP):
                    gs = []
                    for j in range(CPG):
                        c = G * CPG + j
                        g = gpool.tile([128, tiles_per_call, NSLAB], dt.bfloat16)
                        nc.gpsimd.dma_gather(
                            g[:], xt[:],
                            idxs_sb[:, c * (TPC // 16):(c + 1) * (TPC // 16)],
                            TPC, TPC, NSLAB,
                            queue_num=c % 4,
                        )
                        gs.append(g)
                    diag = dpool.tile([128, K, 128], dt.bfloat16)
                    wb = (w_sb[:, G * K:(G + 1) * K]
                          .unsqueeze(2).broadcast_to([128, K, 128]))
                    nc.vector.tensor_tensor(diag[:], ident_b, wb,
                                            op=mybir.AluOpType.mult)
                    p = ppool.tile([128, NSLAB], dt.float32)
                    for m in range(K):
                        gt = gs[m // tiles_per_call][:, m % tiles_per_call, :]
                        for h2 in range(NSLAB // 512):
                            nc.tensor.matmul(
                                p[:, h2 * 512:(h2 + 1) * 512],
                                diag[:, m, :],
                                gt[:, h2 * 512:(h2 + 1) * 512],
                                start=(m == 0), stop=(m == K - 1))
                    o = opool.tile([128, NSLAB], dt.float32)
                    nc.scalar.activation(
                        o[:], p[:], mybir.ActivationFunctionType.Identity,
                        bias=bias_sb[:, G:G + 1])
                    nc.sync.dma_start(outT[G * 128:(G + 1) * 128, :], o[:])

            if reps == 1:
                body()
            else:
                with tc.For_i(0, reps, 1):
                    body()

    nc.compile()
    return nc


def _prep_half(idx_h, w_h, b_h):
    """Per-output-half tap/weight/bias tables (idx_h: (OSLAB, K))."""
    # tap order: flat[(G*16 + m)*128 + p] = idx_h[128G + p, m]
    idx_flat = idx_h.reshape(NGRP, 128, K).transpose(0, 2, 1).reshape(-1)
    wrap = idx_flat.reshape(TAPS // 16, 16).T           # [16, TAPS//16]
    idxs_np = np.tile(wrap, (8, 1)).astype(np.int16)    # [128, TAPS//16]
    wcol_np = (w_h.reshape(NGRP, 128, K).transpose(1, 0, 2)
               .reshape(128, NGRP * K).astype(ml_dtypes.bfloat16))
    bias_np = np.ascontiguousarray(b_h.reshape(NGRP, 128).T)  # [128, NGRP]
    return idxs_np, wcol_np, bias_np


def _prep_inputs(x, in_index_per_out, weight, bias):
    idx = np.asarray(in_index_per_out).astype(np.int64)
    w = np.asarray(weight).astype(np.float32)
    b = np.asarray(bias).astype(np.float32)

    halves = [_prep_half(idx[h * OSLAB:(h + 1) * OSLAB],
                         w[h * OSLAB:(h + 1) * OSLAB],
                         b[h * OSLAB:(h + 1) * OSLAB]) for h in range(OSPLIT)]
    ident_np = np.eye(128, dtype=ml_dtypes.bfloat16)
    xT = np.ascontiguousarray(np.asarray(x).astype(np.float32).T
                              .astype(ml_dtypes.bfloat16))  # (IN_F, N)
    slabs = [np.ascontiguousarray(xT[:, s * NSLAB:(s + 1) * NSLAB])
             for s in range(NSPLIT)]
    return halves, ident_np, slabs


def kernel(x, in_index_per_out, weight, bias):
    from concourse import bass_utils

    halves, ident_np, slabs = _prep_inputs(x, in_index_per_out, weight, bias)

    if "nc" not in _CACHE:
        _CACHE["nc"] = _build(reps=1)
    nc = _CACHE["nc"]

    in_maps = []
    for d in range(NCORES):
        s, h = d % NSPLIT, d // NSPLIT
        idxs_np, wcol_np, bias_np = halves[h]
        in_maps.append({"xt": slabs[s], "idxs": idxs_np, "wcol": wcol_np,
                        "bias": bias_np, "ident": ident_np})
    res = bass_utils.run_bass_kernel_spmd(nc, in_maps,
                                          core_ids=list(range(NCORES)))
    out = np.empty((N, OUT_F), dtype=np.float32)
    for d in range(NCORES):
        s, h = d % NSPLIT, d // NSPLIT
        out[s * NSLAB:(s + 1) * NSLAB, h * OSLAB:(h + 1) * OSLAB] = \
            res.results[d]["outT"].T
    return out
